# revision 4
# baseline (speedup 1.0000x reference)
"""CoAttention kernel for Trainium2, data-parallel over batch across 8 NeuronCores.

Per core (one batch element b):
    query = data1[b] @ Wq + bq                      # [2048, 256]
    key   = data2[b] @ Wk + bk                      # [2048, 256]
    attn  = softmax(SCALE * query @ key_nb^T)       # bias bk cancels in softmax
    out   = attn @ key + query

Device-side layout strategy (all matmuls bf16 inputs, fp32 PSUM accumulate):
  - data1/data2 are cast-loaded (fp32->bf16, SWDGE) and transposed on-chip via
    the DMA xbar so the contraction dim lands on SBUF partitions.
  - scoresT [k, q] = (Wk-proj data2)^T-oriented matmul so exp(scoresT) feeds the
    context matmul directly as the stationary operand.
  - softmax denominator comes for free as a ones-column appended to the key
    value matrix; no max-subtraction is needed (|SCALE*scores| < ~4 for this
    problem scale, exp is exact in fp32 there).
  - residual query is computed in a second projection pass and added in fp32.
"""

import sys

if "/opt/trn_rl_repo" not in sys.path:
    sys.path.insert(0, "/opt/trn_rl_repo")

from contextlib import ExitStack

import numpy as np

import concourse.bass as bass  # noqa: F401  (AP helpers)
import concourse.mybir as mybir
import concourse.tile as tile
from concourse import bacc
from concourse.bass_utils import run_bass_kernel_spmd

B, LQ, LK, DIN, D = 8, 2048, 2048, 1024, 256
N_CORES = 8
SCALE = float(1.0 / np.sqrt(1024.0).astype(np.float32))

BF16 = mybir.dt.bfloat16
F32 = mybir.dt.float32
AF = mybir.ActivationFunctionType


def _build():
    nc = bacc.Bacc("TRN2", target_bir_lowering=False, debug=False)
    d1 = nc.dram_tensor("data1", [LQ, DIN], F32, kind="ExternalInput").ap()
    d2 = nc.dram_tensor("data2", [LK, D], F32, kind="ExternalInput").ap()
    wq = nc.dram_tensor("Wq", [DIN, D], F32, kind="ExternalInput").ap()
    wkx = nc.dram_tensor("Wk_ext", [D, D + 1], F32, kind="ExternalInput").ap()
    bq = nc.dram_tensor("bq", [D], F32, kind="ExternalInput").ap()
    bkx = nc.dram_tensor("bk_ext", [D + 1], F32, kind="ExternalInput").ap()
    out = nc.dram_tensor("out", [LQ, D], F32, kind="ExternalOutput").ap()

    QT_TILES = LQ // 128  # 16
    KT_TILES = LK // 128  # 16
    IC1 = DIN // 128  # 8
    IC2 = D // 128  # 2

    with tile.TileContext(nc) as tc, ExitStack() as ctx:
        const = ctx.enter_context(tc.tile_pool(name="const", bufs=1))
        big = ctx.enter_context(tc.tile_pool(name="big", bufs=1))
        stage1 = ctx.enter_context(tc.tile_pool(name="stage1", bufs=3))
        stage2 = ctx.enter_context(tc.tile_pool(name="stage2", bufs=3))
        small = ctx.enter_context(tc.tile_pool(name="small", bufs=4))
        ps_proj = ctx.enter_context(tc.tile_pool(name="ps_proj", bufs=2, space="PSUM"))
        ps_sc = ctx.enter_context(tc.tile_pool(name="ps_sc", bufs=2, space="PSUM"))
        ps_ctx = ctx.enter_context(tc.tile_pool(name="ps_ctx", bufs=2, space="PSUM"))

        # ---------------- weights / constants ----------------
        wq_sb = [const.tile([128, D], BF16, tag=f"wq{i}", name=f"wq{i}") for i in range(IC1)]
        for i in range(IC1):
            nc.gpsimd.dma_start(out=wq_sb[i][:], in_=wq[i * 128:(i + 1) * 128, :])
        wk_sb = [const.tile([128, D + 1], BF16, tag=f"wk{i}", name=f"wk{i}") for i in range(IC2)]
        for i in range(IC2):
            nc.gpsimd.dma_start(out=wk_sb[i][:], in_=wkx[i * 128:(i + 1) * 128, :])
        bq_row = const.tile([1, D], BF16, tag="bq_row")
        nc.gpsimd.dma_start(out=bq_row[:], in_=bq.rearrange("(a d) -> a d", a=1))
        bkx_row = const.tile([1, D + 1], BF16, tag="bkx_row")
        nc.gpsimd.dma_start(out=bkx_row[:], in_=bkx.rearrange("(a d) -> a d", a=1))
        bq_col = const.tile([128, IC2], F32, tag="bq_col")
        for c in range(IC2):
            nc.sync.dma_start(
                out=bq_col[:, c:c + 1],
                in_=bq[c * 128:(c + 1) * 128].rearrange("(p a) -> p a", a=1),
            )
        ones_row = const.tile([1, 128], BF16, tag="ones_row")
        nc.vector.memset(ones_row[:], 1.0)

        # ---------------- load + transpose data2 ----------------
        d2T = [big.tile([128, LK], BF16, tag=f"d2T{i}", name=f"d2T{i}") for i in range(IC2)]
        for kt in range(KT_TILES):
            st = stage2.tile([128, D], BF16, tag="d2st")
            nc.gpsimd.dma_start(out=st[:], in_=d2[kt * 128:(kt + 1) * 128, :])
            for ic in range(IC2):
                nc.sync.dma_start(
                    out=d2T[ic][:, kt * 128:(kt + 1) * 128],
                    in_=st[:, ic * 128:(ic + 1) * 128],
                    transpose=True,
                )

        # ---------------- load + transpose data1 ----------------
        d1T = [big.tile([128, LQ], BF16, tag=f"d1T{i}", name=f"d1T{i}") for i in range(IC1)]
        for qt in range(QT_TILES):
            st = stage1.tile([128, DIN], BF16, tag="d1st")
            nc.gpsimd.dma_start(out=st[:], in_=d1[qt * 128:(qt + 1) * 128, :])
            for ic in range(IC1):
                nc.sync.dma_start(
                    out=d1T[ic][:, qt * 128:(qt + 1) * 128],
                    in_=st[:, ic * 128:(ic + 1) * 128],
                    transpose=True,
                )

        # ---------------- K^T (no bias; bias cancels in softmax) ----------------
        kt_sb = [big.tile([128, LK], BF16, tag=f"kt{dc}", name=f"kt{dc}") for dc in range(2)]
        for dc in range(2):
            for nk in range(LK // 512):
                ps = ps_proj.tile([128, 512], F32, tag="ps_proj")
                for ic in range(IC2):
                    nc.tensor.matmul(
                        ps[:],
                        lhsT=wk_sb[ic][:, dc * 128:(dc + 1) * 128],
                        rhs=d2T[ic][:, nk * 512:(nk + 1) * 512],
                        start=(ic == 0),
                        stop=(ic == IC2 - 1),
                    )
                nc.vector.tensor_copy(kt_sb[dc][:, nk * 512:(nk + 1) * 512], ps[:])

        # ---------------- key value matrix [k, 257] = [key | 1] ----------------
        key_sb = [big.tile([128, D + 1], BF16, tag=f"key{kb}", name=f"key{kb}") for kb in range(KT_TILES)]
        for kb in range(KT_TILES):
            ps = ps_proj.tile([128, 512], F32, tag="ps_proj")
            p = ps[:, :D + 1]
            for ic in range(IC2):
                nc.tensor.matmul(
                    p,
                    lhsT=d2T[ic][:, kb * 128:(kb + 1) * 128],
                    rhs=wk_sb[ic][:],
                    start=(ic == 0),
                    stop=False,
                )
            nc.tensor.matmul(p, lhsT=ones_row[:], rhs=bkx_row[:], start=False, stop=True)
            nc.vector.tensor_copy(key_sb[kb][:], p)

        # ---------------- Q^T (with bias, per-partition via activation) ----------------
        qt_sb = [big.tile([128, LQ], BF16, tag=f"qt{dc}", name=f"qt{dc}") for dc in range(2)]
        for dc in range(2):
            for nq in range(LQ // 512):
                ps = ps_proj.tile([128, 512], F32, tag="ps_proj")
                for ic in range(IC1):
                    nc.tensor.matmul(
                        ps[:],
                        lhsT=wq_sb[ic][:, dc * 128:(dc + 1) * 128],
                        rhs=d1T[ic][:, nq * 512:(nq + 1) * 512],
                        start=(ic == 0),
                        stop=(ic == IC1 - 1),
                    )
                nc.scalar.activation(
                    qt_sb[dc][:, nq * 512:(nq + 1) * 512], ps[:], AF.Identity,
                    bias=bq_col[:, dc:dc + 1],
                )

        # ---------------- Q residual [q, d] fp32 ----------------
        q_sb = [big.tile([128, D], F32, tag=f"q{qb}", name=f"q{qb}") for qb in range(QT_TILES)]
        for qb in range(QT_TILES):
            ps = ps_proj.tile([128, 512], F32, tag="ps_proj")
            p = ps[:, :D]
            for ic in range(IC1):
                nc.tensor.matmul(
                    p,
                    lhsT=d1T[ic][:, qb * 128:(qb + 1) * 128],
                    rhs=wq_sb[ic][:],
                    start=(ic == 0),
                    stop=False,
                )
            nc.tensor.matmul(p, lhsT=ones_row[:], rhs=bq_row[:], start=False, stop=True)
            nc.vector.tensor_copy(q_sb[qb][:], p)

        # ---------------- scores^T -> exp ----------------
        # expT[km][nh] covers k in [km*128, +128), q in [nh*1024, +1024)
        expT = [
            [big.tile([128, 1024], BF16, tag=f"expT{km}_{nh}", name=f"expT{km}_{nh}") for nh in range(2)]
            for km in range(KT_TILES)
        ]
        for km in range(KT_TILES):
            for nh in range(2):
                ps = ps_sc.tile([128, 1024], F32, tag="ps_sc")
                for half in range(2):
                    nq = nh * 2 + half
                    for dc in range(2):
                        nc.tensor.matmul(
                            ps[:, half * 512:(half + 1) * 512],
                            lhsT=kt_sb[dc][:, km * 128:(km + 1) * 128],
                            rhs=qt_sb[dc][:, nq * 512:(nq + 1) * 512],
                            start=(dc == 0),
                            stop=(dc == 1),
                        )
                nc.scalar.activation(expT[km][nh][:], ps[:], AF.Exp, scale=SCALE)

        # ---------------- context + normalize + residual ----------------
        out_sb = big.tile([128, QT_TILES * D], F32, tag="out_sb")
        for qb in range(QT_TILES):
            pc = ps_ctx.tile([128, D + 1], F32, tag="ps_ctx")
            for km in range(KT_TILES):
                nc.tensor.matmul(
                    pc[:],
                    lhsT=expT[km][qb // 8][:, (qb % 8) * 128:(qb % 8 + 1) * 128],
                    rhs=key_sb[km][:],
                    start=(km == 0),
                    stop=(km == KT_TILES - 1),
                )
            rc = small.tile([128, 1], F32, tag="recip")
            nc.vector.reciprocal(rc[:], pc[:, D:D + 1])
            osl = out_sb[:, qb * D:(qb + 1) * D]
            nc.vector.tensor_scalar(osl, pc[:, :D], rc[:], None, mybir.AluOpType.mult)
            nc.vector.tensor_add(osl, osl, q_sb[qb][:])

        nc.sync.dma_start(
            out=out.rearrange("(qt p) d -> p qt d", p=128),
            in_=out_sb[:].rearrange("p (qt d) -> p qt d", d=D),
        )

    nc.compile()
    return nc


_NC = None


def _get_nc():
    global _NC
    if _NC is None:
        _NC = _build()
    return _NC


def kernel(data1, data2, Wq, bq, Wk, bk):
    data1 = np.asarray(data1, dtype=np.float32)
    data2 = np.asarray(data2, dtype=np.float32)
    Wq = np.ascontiguousarray(np.asarray(Wq, dtype=np.float32))
    bq = np.ascontiguousarray(np.asarray(bq, dtype=np.float32))
    Wk = np.asarray(Wk, dtype=np.float32)
    bk = np.asarray(bk, dtype=np.float32)

    wk_ext = np.zeros((D, D + 1), dtype=np.float32)
    wk_ext[:, :D] = Wk
    bk_ext = np.concatenate([bk, np.ones(1, dtype=np.float32)]).astype(np.float32)

    nc = _get_nc()
    in_maps = [
        {
            "data1": np.ascontiguousarray(data1[b]),
            "data2": np.ascontiguousarray(data2[b]),
            "Wq": Wq,
            "Wk_ext": wk_ext,
            "bq": bq,
            "bk_ext": bk_ext,
        }
        for b in range(B)
    ]
    res = run_bass_kernel_spmd(nc, in_maps, core_ids=list(range(N_CORES)))
    return np.stack([res.results[i]["out"] for i in range(B)], axis=0)


# revision 6
# speedup vs baseline: 2.6052x; 2.6052x over previous
"""CoAttention kernel for Trainium2, data-parallel over batch across 8 NeuronCores.

Per core (one batch element b):
    query = data1[b] @ Wq + bq                      # [2048, 256]
    key   = data2[b] @ Wk + bk                      # [2048, 256]
    attn  = softmax(SCALE * query @ key_nb^T)       # bias bk cancels in softmax
    out   = attn @ key + query

Device-side layout strategy (all matmuls bf16 inputs, fp32 PSUM accumulate):
  - data1/data2 are cast-loaded (fp32->bf16, SWDGE) and transposed on-chip via
    the DMA xbar so the contraction dim lands on SBUF partitions.
  - scoresT [k, q] = (Wk-proj data2)^T-oriented matmul so exp(scoresT) feeds the
    context matmul directly as the stationary operand.
  - softmax denominator comes for free as a ones-column appended to the key
    value matrix; no max-subtraction is needed (|SCALE*scores| < ~4 for this
    problem scale, exp is exact in fp32 there).
  - residual query is computed in a second projection pass and added in fp32.
"""

import sys

if "/opt/trn_rl_repo" not in sys.path:
    sys.path.insert(0, "/opt/trn_rl_repo")

from contextlib import ExitStack

import numpy as np

import concourse.bass as bass  # noqa: F401  (AP helpers)
import concourse.mybir as mybir
import concourse.tile as tile
from concourse import bacc
from concourse.bass_utils import run_bass_kernel_spmd

B, LQ, LK, DIN, D = 8, 2048, 2048, 1024, 256
N_CORES = 8
SCALE = float(1.0 / np.sqrt(1024.0).astype(np.float32))

BF16 = mybir.dt.bfloat16
F32 = mybir.dt.float32
AF = mybir.ActivationFunctionType


def _build():
    nc = bacc.Bacc("TRN2", target_bir_lowering=False, debug=False)
    d1 = nc.dram_tensor("data1", [LQ, DIN], F32, kind="ExternalInput").ap()
    d2 = nc.dram_tensor("data2", [LK, D], F32, kind="ExternalInput").ap()
    wq = nc.dram_tensor("Wq", [DIN, D], F32, kind="ExternalInput").ap()
    wkx = nc.dram_tensor("Wk_ext", [D, D + 1], F32, kind="ExternalInput").ap()
    bq = nc.dram_tensor("bq", [D], F32, kind="ExternalInput").ap()
    bkx = nc.dram_tensor("bk_ext", [D + 1], F32, kind="ExternalInput").ap()
    out = nc.dram_tensor("out", [LQ, D], F32, kind="ExternalOutput").ap()

    QT_TILES = LQ // 128  # 16
    KT_TILES = LK // 128  # 16
    IC1 = DIN // 128  # 8
    IC2 = D // 128  # 2

    with tile.TileContext(nc) as tc, ExitStack() as ctx:
        const = ctx.enter_context(tc.tile_pool(name="const", bufs=1))
        big = ctx.enter_context(tc.tile_pool(name="big", bufs=1))
        dram = ctx.enter_context(tc.tile_pool(name="dram", bufs=1, space="DRAM"))
        small = ctx.enter_context(tc.tile_pool(name="small", bufs=4))
        ps_proj = ctx.enter_context(tc.tile_pool(name="ps_proj", bufs=2, space="PSUM"))
        ps_sc = ctx.enter_context(tc.tile_pool(name="ps_sc", bufs=2, space="PSUM"))
        ps_ctx = ctx.enter_context(tc.tile_pool(name="ps_ctx", bufs=2, space="PSUM"))

        # ---------------- weights / constants ----------------
        wq_sb = [const.tile([128, D], BF16, tag=f"wq{i}", name=f"wq{i}") for i in range(IC1)]
        for i in range(IC1):
            nc.gpsimd.dma_start(out=wq_sb[i][:], in_=wq[i * 128:(i + 1) * 128, :])
        wk_sb = [const.tile([128, D + 1], BF16, tag=f"wk{i}", name=f"wk{i}") for i in range(IC2)]
        for i in range(IC2):
            nc.gpsimd.dma_start(out=wk_sb[i][:], in_=wkx[i * 128:(i + 1) * 128, :])
        bq_row = const.tile([1, D], BF16, tag="bq_row")
        nc.gpsimd.dma_start(out=bq_row[:], in_=bq.rearrange("(a d) -> a d", a=1))
        bkx_row = const.tile([1, D + 1], BF16, tag="bkx_row")
        nc.gpsimd.dma_start(out=bkx_row[:], in_=bkx.rearrange("(a d) -> a d", a=1))
        bq_col = const.tile([128, IC2], F32, tag="bq_col")
        for c in range(IC2):
            nc.sync.dma_start(
                out=bq_col[:, c:c + 1],
                in_=bq[c * 128:(c + 1) * 128].rearrange("(p a) -> p a", a=1),
            )
        ones_row = const.tile([1, 128], BF16, tag="ones_row")
        nc.vector.memset(ones_row[:], 1.0)

        # ---------------- load + transpose data2 ----------------
        # fp32 -> bf16 cast to a DRAM scratch (SWDGE), then one big xbar
        # transpose per 128-wide column chunk straight from DRAM.
        d2bf = dram.tile([LK, D], BF16, tag="d2bf", name="d2bf")
        nc.gpsimd.dma_start(out=d2bf[:], in_=d2[:])
        d2T = [big.tile([128, LK], BF16, tag=f"d2T{i}", name=f"d2T{i}") for i in range(IC2)]
        for ic in range(IC2):
            nc.sync.dma_start(
                out=d2T[ic][:],
                in_=d2bf[:, ic * 128:(ic + 1) * 128],
                transpose=True,
            )

        # ---------------- load + transpose data1 (2 halves for overlap) ----------------
        NH1 = 2
        HQ = LQ // NH1
        d1bf = [dram.tile([HQ, DIN], BF16, tag=f"d1bf{h}", name=f"d1bf{h}") for h in range(NH1)]
        d1T = [big.tile([128, LQ], BF16, tag=f"d1T{i}", name=f"d1T{i}") for i in range(IC1)]
        for h in range(NH1):
            nc.gpsimd.dma_start(out=d1bf[h][:], in_=d1[h * HQ:(h + 1) * HQ, :])
            for ic in range(IC1):
                nc.sync.dma_start(
                    out=d1T[ic][:, h * HQ:(h + 1) * HQ],
                    in_=d1bf[h][:, ic * 128:(ic + 1) * 128],
                    transpose=True,
                )

        # ---------------- K^T (no bias; bias cancels in softmax) ----------------
        kt_sb = [big.tile([128, LK], BF16, tag=f"kt{dc}", name=f"kt{dc}") for dc in range(2)]
        for dc in range(2):
            for nk in range(LK // 512):
                ps = ps_proj.tile([128, 512], F32, tag="ps_proj")
                for ic in range(IC2):
                    nc.tensor.matmul(
                        ps[:],
                        lhsT=wk_sb[ic][:, dc * 128:(dc + 1) * 128],
                        rhs=d2T[ic][:, nk * 512:(nk + 1) * 512],
                        start=(ic == 0),
                        stop=(ic == IC2 - 1),
                    )
                nc.vector.tensor_copy(kt_sb[dc][:, nk * 512:(nk + 1) * 512], ps[:])

        # ---------------- key value matrix [k, 257] = [key | 1] ----------------
        key_sb = [big.tile([128, D + 1], BF16, tag=f"key{kb}", name=f"key{kb}") for kb in range(KT_TILES)]
        for kb in range(KT_TILES):
            ps = ps_proj.tile([128, 512], F32, tag="ps_proj")
            p = ps[:, :D + 1]
            for ic in range(IC2):
                nc.tensor.matmul(
                    p,
                    lhsT=d2T[ic][:, kb * 128:(kb + 1) * 128],
                    rhs=wk_sb[ic][:],
                    start=(ic == 0),
                    stop=False,
                )
            nc.tensor.matmul(p, lhsT=ones_row[:], rhs=bkx_row[:], start=False, stop=True)
            nc.vector.tensor_copy(key_sb[kb][:], p)

        # ---------------- Q^T (with bias, per-partition via activation) ----------------
        qt_sb = [big.tile([128, LQ], BF16, tag=f"qt{dc}", name=f"qt{dc}") for dc in range(2)]
        for dc in range(2):
            for nq in range(LQ // 512):
                ps = ps_proj.tile([128, 512], F32, tag="ps_proj")
                for ic in range(IC1):
                    nc.tensor.matmul(
                        ps[:],
                        lhsT=wq_sb[ic][:, dc * 128:(dc + 1) * 128],
                        rhs=d1T[ic][:, nq * 512:(nq + 1) * 512],
                        start=(ic == 0),
                        stop=(ic == IC1 - 1),
                    )
                nc.scalar.activation(
                    qt_sb[dc][:, nq * 512:(nq + 1) * 512], ps[:], AF.Identity,
                    bias=bq_col[:, dc:dc + 1],
                )

        # ---------------- Q residual [q, d] fp32 ----------------
        q_sb = [big.tile([128, D], F32, tag=f"q{qb}", name=f"q{qb}") for qb in range(QT_TILES)]
        for qb in range(QT_TILES):
            ps = ps_proj.tile([128, 512], F32, tag="ps_proj")
            p = ps[:, :D]
            for ic in range(IC1):
                nc.tensor.matmul(
                    p,
                    lhsT=d1T[ic][:, qb * 128:(qb + 1) * 128],
                    rhs=wq_sb[ic][:],
                    start=(ic == 0),
                    stop=False,
                )
            nc.tensor.matmul(p, lhsT=ones_row[:], rhs=bq_row[:], start=False, stop=True)
            nc.vector.tensor_copy(q_sb[qb][:], p)

        # ---------------- scores^T -> exp ----------------
        # expT[km][nh] covers k in [km*128, +128), q in [nh*1024, +1024)
        expT = [
            [big.tile([128, 1024], BF16, tag=f"expT{km}_{nh}", name=f"expT{km}_{nh}") for nh in range(2)]
            for km in range(KT_TILES)
        ]
        for km in range(KT_TILES):
            for nh in range(2):
                ps = ps_sc.tile([128, 1024], F32, tag="ps_sc")
                for half in range(2):
                    nq = nh * 2 + half
                    for dc in range(2):
                        nc.tensor.matmul(
                            ps[:, half * 512:(half + 1) * 512],
                            lhsT=kt_sb[dc][:, km * 128:(km + 1) * 128],
                            rhs=qt_sb[dc][:, nq * 512:(nq + 1) * 512],
                            start=(dc == 0),
                            stop=(dc == 1),
                        )
                nc.scalar.activation(expT[km][nh][:], ps[:], AF.Exp, scale=SCALE)

        # ---------------- context + normalize + residual ----------------
        out_sb = big.tile([128, QT_TILES * D], F32, tag="out_sb")
        for qb in range(QT_TILES):
            pc = ps_ctx.tile([128, D + 1], F32, tag="ps_ctx")
            for km in range(KT_TILES):
                nc.tensor.matmul(
                    pc[:],
                    lhsT=expT[km][qb // 8][:, (qb % 8) * 128:(qb % 8 + 1) * 128],
                    rhs=key_sb[km][:],
                    start=(km == 0),
                    stop=(km == KT_TILES - 1),
                )
            rc = small.tile([128, 1], F32, tag="recip")
            nc.vector.reciprocal(rc[:], pc[:, D:D + 1])
            osl = out_sb[:, qb * D:(qb + 1) * D]
            nc.vector.tensor_scalar(osl, pc[:, :D], rc[:], None, mybir.AluOpType.mult)
            nc.vector.tensor_add(osl, osl, q_sb[qb][:])

        nc.sync.dma_start(
            out=out.rearrange("(qt p) d -> p qt d", p=128),
            in_=out_sb[:].rearrange("p (qt d) -> p qt d", d=D),
        )

    nc.compile()
    return nc


_NC = None


def _get_nc():
    global _NC
    if _NC is None:
        _NC = _build()
    return _NC


def kernel(data1, data2, Wq, bq, Wk, bk):
    data1 = np.asarray(data1, dtype=np.float32)
    data2 = np.asarray(data2, dtype=np.float32)
    Wq = np.ascontiguousarray(np.asarray(Wq, dtype=np.float32))
    bq = np.ascontiguousarray(np.asarray(bq, dtype=np.float32))
    Wk = np.asarray(Wk, dtype=np.float32)
    bk = np.asarray(bk, dtype=np.float32)

    wk_ext = np.zeros((D, D + 1), dtype=np.float32)
    wk_ext[:, :D] = Wk
    bk_ext = np.concatenate([bk, np.ones(1, dtype=np.float32)]).astype(np.float32)

    nc = _get_nc()
    in_maps = [
        {
            "data1": np.ascontiguousarray(data1[b]),
            "data2": np.ascontiguousarray(data2[b]),
            "Wq": Wq,
            "Wk_ext": wk_ext,
            "bq": bq,
            "bk_ext": bk_ext,
        }
        for b in range(B)
    ]
    res = run_bass_kernel_spmd(nc, in_maps, core_ids=list(range(N_CORES)))
    return np.stack([res.results[i]["out"] for i in range(B)], axis=0)
